# revision 1
# baseline (speedup 1.0000x reference)
"""Fused global pooling (mean/max/std over H*W per channel) + tiny MLP.

Input x: [1024, 1024, 384] f32. Sharded along H across 8 NeuronCores
(128 H-rows each). Each core computes per-channel partial sum / sumsq
(via ones-matmul on PE, f32r) and per-channel max (DVE tree-max into a
384-wide accumulator); host combines the 8 partial stats, finishes
mean/std/max, and runs the 3-layer MLP.

Tail latency is minimized: the trailing tiles are streamed as
progressively smaller sub-tiles (TAILCFG) so the serial DVE max chain
tracks DMA arrivals and almost no work remains after the last byte
lands; PSUM->SBUF copies + the combined sum/sumsq output DMA run on the
Scalar engine while out_max goes out on Sync.
"""
import os
import sys

sys.path.insert(0, "/opt/trn_rl_repo")

import numpy as np

H, W, C = 1024, 1024, 384
N_CORES = 8
P = 128                      # SBUF partitions; also H-rows per core
ROW = W * C                  # elements per H-row = 393216
FD = int(os.environ.get("BASS_KERNEL_FD", "6144"))  # free elems/tile/partition
# Sub-split widths for the trailing tiles (innermost list = one tile).
# Finer DMA granularity near the end lets the serial DVE max chain start
# each piece earlier, so almost nothing remains after the last byte lands.
TAILCFG = os.environ.get(
    "BASS_KERNEL_TAILCFG",
    "3072,3072;3072,3072;3072,3072;3072,3072;"
    "1536,1536,1536,1536;1536,1536,1536,768,768",
)
XBUFS = int(os.environ.get("BASS_KERNEL_XBUFS", "3"))
SQBUFS = int(os.environ.get("BASS_KERNEL_SQBUFS", "2"))
TMPBUFS = int(os.environ.get("BASS_KERNEL_TMPBUFS", "2"))
ALTQ = bool(int(os.environ.get("BASS_KERNEL_ALTQ", "0")))  # alternate HWDGE queues

_CACHE: dict = {}

# set by test.py via env to collect a perfetto trace + HW exec time
TRACE = bool(int(os.environ.get("BASS_KERNEL_TRACE", "0")))
last_result = None           # BassKernelResults of the most recent run


def _build():
    import concourse.bacc as bacc
    import concourse.mybir as mybir
    import concourse.tile as tile

    T = ROW // FD            # tiles per core
    R = FD // C              # channel-groups per tile
    tailcfg = [
        [int(w) for w in tile_spec.split(",")]
        for tile_spec in TAILCFG.split(";")
    ]
    assert T * FD == ROW and R * C == FD
    for ws in tailcfg:
        assert sum(ws) == FD and all(w % C == 0 for w in ws)
    MSPLIT = len(tailcfg)

    # square-chunk width: bound the sq tile at <= 6144 elems/partition
    SQW = min(FD, 6144)
    NSQ = FD // SQW

    f32 = mybir.dt.float32
    f32r = mybir.dt.float32r

    nc = bacc.Bacc(trn_type="TRN2")
    x_in = nc.declare_dram_parameter("x", [P, ROW], f32, isOutput=False)
    out_max = nc.declare_dram_parameter("out_max", [P, C], f32, isOutput=True)
    out_stats = nc.declare_dram_parameter("out_stats", [1, 2 * C], f32, isOutput=True)

    ACCW = int(os.environ.get("BASS_KERNEL_ACCW", "1536"))  # max-acc width

    def tree_max(xt, tmp, acc, lo, hi):
        """acc[:, 0:ACCW] = max(acc, fold(xt[:, lo:hi])) via log-halving.

        Total DVE element work is (hi-lo) regardless of ACCW; a wider
        accumulator means fewer instructions per tile (more DVE slack vs
        the DMA cadence) at the cost of a slightly longer final fold."""
        width = hi - lo
        if width <= ACCW:
            # sub-tile no wider than the accumulator: merge directly
            a = lo % ACCW
            nc.vector.tensor_max(acc[:, a:a + width], acc[:, a:a + width],
                                 xt[:, lo:hi].bitcast(f32))
            return
        w = width // 2
        nc.vector.tensor_max(tmp[:, 0:w], xt[:, lo:lo + w].bitcast(f32),
                             xt[:, lo + w:hi].bitcast(f32))
        while w > ACCW:
            half = w // 2
            nc.vector.tensor_max(tmp[:, 0:half], tmp[:, 0:half],
                                 tmp[:, half:w])
            w = half
        nc.vector.tensor_max(acc[:], acc[:], tmp[:, 0:ACCW])

    with tile.TileContext(nc) as tc:
        with (
            tc.tile_pool(name="x", bufs=XBUFS) as xpool,
            tc.tile_pool(name="sq", bufs=SQBUFS) as sqpool,
            tc.tile_pool(name="tmp", bufs=TMPBUFS) as tmppool,
            tc.tile_pool(name="acc", bufs=1) as accpool,
            tc.tile_pool(name="misc", bufs=1) as misc,
            tc.tile_pool(name="psum", bufs=1, space="PSUM") as psum_pool,
        ):
            ones_f = misc.tile([P, 1], f32, tag="ones_f")
            nc.vector.memset(ones_f[:], 1.0)
            ones = misc.tile([P, 1], f32r, tag="ones")
            nc.vector.tensor_copy(ones[:], ones_f[:])

            acc = accpool.tile([P, ACCW], f32)
            nc.vector.memset(acc[:], float("-inf"))

            ps_sum = psum_pool.tile([1, C], f32)
            ps_sq = psum_pool.tile([1, C], f32)

            dmaq = [nc.sync, nc.scalar] if ALTQ else [nc.sync]

            for t in range(T - MSPLIT):
                xt = xpool.tile([P, FD], f32r)
                dmaq[t % len(dmaq)].dma_start(
                    xt[:], x_in[:, t * FD:(t + 1) * FD].bitcast(f32r)
                )

                for h in range(NSQ):
                    hsl = slice(h * SQW, (h + 1) * SQW)
                    sq = sqpool.tile([P, SQW], f32r, tag="sq")
                    nc.scalar.square(sq[:], xt[:, hsl].bitcast(f32))
                    for r in range(SQW // C):
                        g = h * (SQW // C) + r
                        st = (t == 0) and (g == 0)
                        nc.tensor.matmul(
                            ps_sum[:], ones[:], xt[:, g * C:(g + 1) * C],
                            start=st, stop=False,
                        )
                        nc.tensor.matmul(
                            ps_sq[:], ones[:], sq[:, r * C:(r + 1) * C],
                            start=st, stop=False,
                        )
                tmp = tmppool.tile([P, FD // 2], f32, tag="tmp")
                tree_max(xt, tmp, acc, 0, FD)

            # trailing tiles: streamed as sub-tiles per TAILCFG so the serial
            # DVE max chain tracks arrivals and almost no max work remains
            # after the last DMA byte lands
            for i, widths in enumerate(tailcfg):
                t = T - MSPLIT + i
                xt = xpool.tile([P, FD], f32r)
                lo = 0
                for s, wsub in enumerate(widths):
                    hi = lo + wsub
                    dmaq[s % len(dmaq)].dma_start(
                        xt[:, lo:hi], x_in[:, t * FD + lo:t * FD + hi].bitcast(f32r)
                    )
                    sq = sqpool.tile([P, wsub], f32r, tag="sqlast", bufs=4,
                                     padded_shape=[P, FD // 2])
                    nc.scalar.square(sq[:], xt[:, lo:hi].bitcast(f32))
                    last = (i == MSPLIT - 1) and (s == len(widths) - 1)
                    rs = wsub // C
                    for r in range(rs):
                        nc.tensor.matmul(
                            ps_sum[:], ones[:], xt[:, lo + r * C:lo + (r + 1) * C],
                            start=False, stop=last and (r == rs - 1),
                        )
                    for r in range(rs):
                        nc.tensor.matmul(
                            ps_sq[:], ones[:], sq[:, r * C:(r + 1) * C],
                            start=False, stop=last and (r == rs - 1),
                        )
                    tmp = tmppool.tile([P, max(wsub // 2, C)], f32, tag="tmp")
                    tree_max(xt, tmp, acc, lo, hi)
                    lo = hi

            # tail: combined stats copy + DMA entirely on the Scalar engine
            # (no cross-engine hop, doesn't queue behind DVE tree-max work);
            # out_max goes out on Sync as soon as the DVE fold is done.
            w = ACCW // 2
            while w >= C:
                nc.vector.tensor_max(acc[:, 0:w], acc[:, 0:w], acc[:, w:2 * w])
                w //= 2
            stats = misc.tile([1, 2 * C], f32, tag="stats")
            nc.scalar.copy(stats[:, 0:C], ps_sum[:])
            nc.scalar.copy(stats[:, C:2 * C], ps_sq[:])
            nc.sync.dma_start(out_max[:], acc[:, 0:C])
            nc.scalar.dma_start(out_stats[:], stats[:])

    nc.compile()
    return nc


def kernel(x, W1, b1, W2, b2, W3, b3):
    global last_result
    from concourse.bass_utils import run_bass_kernel_spmd

    if "nc" not in _CACHE:
        _CACHE["nc"] = _build()
    nc = _CACHE["nc"]

    x = np.ascontiguousarray(np.asarray(x, dtype=np.float32))
    assert x.shape == (H, W, C)

    core_ids = list(range(N_CORES))
    in_maps = [
        {"x": x[k * P:(k + 1) * P].reshape(P, ROW)} for k in core_ids
    ]
    res = run_bass_kernel_spmd(nc, in_maps, core_ids, trace=TRACE)
    last_result = res

    n = H * W
    sums = np.zeros(C, dtype=np.float64)
    sqs = np.zeros(C, dtype=np.float64)
    mx = np.full(C, -np.inf, dtype=np.float64)
    for k in core_ids:
        r = res.results[k]
        st = r["out_stats"][0].astype(np.float64)
        sums += st[0:C]
        sqs += st[C:2 * C]
        mx = np.maximum(mx, r["out_max"].astype(np.float64).max(axis=0))

    mean = sums / n
    var = (sqs - n * mean * mean) / (n - 1)
    std = np.sqrt(np.maximum(var, 0.0))

    feats = np.concatenate([mean, mx, std])
    h = np.maximum(feats @ np.asarray(W1, np.float64) + np.asarray(b1, np.float64), 0.0)
    h = np.maximum(h @ np.asarray(W2, np.float64) + np.asarray(b2, np.float64), 0.0)
    logits = h @ np.asarray(W3, np.float64) + np.asarray(b3, np.float64)
    e = np.exp(logits - logits.max())
    return (e / e.sum()).astype(np.float32)

